# revision 34
# baseline (speedup 1.0000x reference)
"""Multi-head attention (query-axis softmax variant) on 8 Trainium2 NeuronCores.

Problem: B=4, T=2048, C=1024, H=16, Dh=64.
  q/k/v = per-head projections of x; wei = (q k^T) * C**-0.5, causal-masked;
  softmax over the QUERY axis (axis=2 of (B,H,T,S)); out = attn @ v, concat
  heads, project with Wp and add bp.

Sharding: 8 cores = 4 batches x 2 head-groups (8 heads each).  Each core
computes a partial projection output for its batch; host sums the two
group partials per batch and adds the bias.

Per-core dataflow is fully "transposed" (features on partitions, tokens on
the free axis) so the query-axis softmax stats become free-axis reductions.
Score blocks rotate through pipelined PSUM slots (A = [P,2,1024], its second
half, and B = [P,2,512]) so the next block's matmuls run while the previous
block's exp drains.  Exps are merged two-head 3D-AP ACTIVATEs whose
accumulator yields zh0+zh1 per partition (both heads share the same s on a
partition); h0's Z comes from one DVE reduce of the retained P row and h1's
by subtraction on gpsimd.  attout accumulates per 512-col chunk over the
retained P rows with col-group-paired matmuls; projections (v, q/k of the
next pair, output) fill the PE between score bursts.
"""
import numpy as np

T = 2048
C = 1024
H = 16
DH = 64
B = 4
SCALE = float(C) ** -0.5
NEG = -1e30
P = 128
NCT = 8       # c-tiles (contraction tiles of 128 over C)
NST = 16      # s-tiles of 128 over T

_CACHE = {}


def _build_nc():
    import concourse.bacc as bacc
    import concourse.tile as tile
    import concourse.mybir as mybir

    FP = mybir.dt.float32
    BF = mybir.dt.bfloat16
    AX = mybir.AxisListType.X
    EXP = mybir.ActivationFunctionType.Exp

    nc = bacc.Bacc("TRN2", target_bir_lowering=False, debug=False, num_devices=8)

    xT_d = nc.declare_dram_parameter("xt", [C, T], BF, isOutput=False)
    wq_d = nc.declare_dram_parameter("wq", [C, 512], BF, isOutput=False)
    wk_d = nc.declare_dram_parameter("wk", [C, 512], BF, isOutput=False)
    wv_d = nc.declare_dram_parameter("wv", [C, 512], BF, isOutput=False)
    wp_d = nc.declare_dram_parameter("wpt", [512, C], BF, isOutput=False)
    trif_d = nc.declare_dram_parameter("trif", [P, 256], FP, isOutput=False)
    y_d = nc.declare_dram_parameter("y", [T, C], BF, isOutput=True)

    def blocks(i):
        # all score blocks are <= 512 wide; they rotate through three
        # [P, 2, 512] psum slot units for pipeline depth 3 between a block's
        # exp and later blocks' matmuls
        t0 = P * i
        out = []
        lo = t0
        while lo < T:
            hi = min(lo + 512 - lo % 512 if lo % 512 else lo + 512, T)
            out.append((lo, hi))
            lo = hi
        return out

    with tile.TileContext(nc) as tc:
        with (
            tc.tile_pool(name="perm", bufs=1) as perm,
            tc.tile_pool(name="wst", bufs=2) as wst,
            tc.tile_pool(name="prp", bufs=1) as prp,
            tc.tile_pool(name="st", bufs=6) as st,
            tc.tile_pool(name="ytp", bufs=2) as ytp,
            tc.tile_pool(name="sca", bufs=1, space="PSUM") as sca,
            tc.tile_pool(name="scb", bufs=1, space="PSUM") as scb,
            tc.tile_pool(name="gp", bufs=2, space="PSUM") as gp,
        ):
            warm = perm.tile([P, 512], BF, tag="warm")
            nc.vector.memset(warm[:], 0.0)
            dummy = perm.tile([P, 1], BF, tag="dummy")
            # preload the exp table set off the critical path
            nc.scalar.activation(dummy[:], warm[:, 0:1], EXP, scale=1.0)

            wqt = {}
            wkt = {}

            def stage_w(pn):
                wqt[pn] = wst.tile([P, NCT, P], BF, tag="wq", name=f"wq{pn}")
                wkt[pn] = wst.tile([P, NCT, P], BF, tag="wk", name=f"wk{pn}")
                nc.sync.dma_start(
                    wqt[pn][:],
                    wq_d.ap()[:, P * pn:P * pn + P].rearrange("(a c) m -> c a m", c=P))
                nc.sync.dma_start(
                    wkt[pn][:],
                    wk_d.ap()[:, P * pn:P * pn + P].rearrange("(a c) m -> c a m", c=P))

            stage_w(0)
            xT = perm.tile([P, NCT, T], BF, tag="xT")
            for ct in range(NCT):
                for gq in range(4):
                    nc.sync.dma_start(
                        xT[:, ct, 512 * gq:512 * gq + 512],
                        xT_d.ap()[P * ct:P * ct + P, 512 * gq:512 * gq + 512])
            trif = perm.tile([P, 2, 128], FP, tag="trif")
            nc.sync.dma_start(trif[:], trif_d.ap().rearrange("p (a c) -> p a c", a=2))
            wv = perm.tile([P, NCT, 512], BF, tag="wv")
            nc.sync.dma_start(wv[:], wv_d.ap().rearrange("(a c) m -> c a m", c=P))
            wpt = perm.tile([P, 4, C], BF, tag="wpt")
            nc.sync.dma_start(wpt[:], wp_d.ap().rearrange("(a c) m -> c a m", c=P))

            q_sl = perm.tile([P, 2, T], BF, tag="q")
            k_sl = perm.tile([P, 2, T], BF, tag="k")
            v_sb = perm.tile([P, NST, 512], BF, tag="v")
            ao = perm.tile([P, 4, T], BF, tag="ao")
            vp = perm.tile([P, 2, NST, 128], BF, tag="vp")

            # score slots: allocated once, rotation handled by subtile deps
            scA = sca.tile([P, 2, 1024], FP, tag="sca", name="scA")
            scB = scb.tile([P, 2, 512], FP, tag="scb", name="scB")

            def slot_view(slot, bw):
                if slot == 0:
                    return scA[:, :, 0:bw]
                if slot == 1:
                    return scA[:, :, 512:512 + bw]
                return scB[:, :, 0:bw]

            slot_cur = [0]

            def next_slot(bw):
                s = slot_cur[0]
                slot_cur[0] = (s + 1) % 3
                return slot_view(s, bw)

            # --- warmup junk (PE clock gate) while first DMAs land ---
            for _ in range(12):
                nc.tensor.matmul(scA[:, 0, 0:512], lhsT=warm[:, :P], rhs=warm[:],
                                 start=True, stop=True)

            # --- prefix: q of pair 0 (all 4 groups) + k group 0 only; the
            # remaining k groups become pair-0's first fillers (k group g is
            # first needed as lhsT at s-tile 4g)
            gA = gp.tile([P, 512], FP, tag="g", name="pre_q0")
            gB = gp.tile([P, 512], FP, tag="g", name="pre_q1")
            qdst = [gA[:, :], gB[:, :], scB[:, 0, :], scB[:, 1, :]]
            kdst0 = scA[:, 0, 0:512]
            for ct in range(NCT):
                for gi in range(4):
                    nc.tensor.matmul(
                        qdst[gi], lhsT=wqt[0][:, ct, :],
                        rhs=xT[:, ct, 512 * gi:512 * gi + 512],
                        start=(ct == 0), stop=(ct == NCT - 1))
                nc.tensor.matmul(
                    kdst0, lhsT=wkt[0][:, ct, :], rhs=xT[:, ct, 0:512],
                    start=(ct == 0), stop=(ct == NCT - 1))
            nc.vector.tensor_copy(k_sl[:, 0, 0:512], kdst0)
            for gi in range(4):
                nc.vector.tensor_copy(q_sl[:, 0, 512 * gi:512 * gi + 512], qdst[gi])

            # --- filler generators: each next() emits a ~0.85us quantum of PE
            # work (4 matmuls) so the PE interleave tracks the exp cadence ---
            def qk_group(pn, which, gi):
                gt = gp.tile([P, 512], FP, tag="g", name=f"{which}{pn}_{gi}")
                wt = wqt[pn] if which == "q" else wkt[pn]
                dst = q_sl if which == "q" else k_sl
                for ct in range(NCT):
                    nc.tensor.matmul(
                        gt[:], lhsT=wt[:, ct, :],
                        rhs=xT[:, ct, 512 * gi:512 * gi + 512],
                        start=(ct == 0), stop=(ct == NCT - 1))
                    if ct == 3:
                        yield
                nc.vector.tensor_copy(dst[:, pn % 2, 512 * gi:512 * gi + 512], gt[:])
                yield

            def v_group(g):
                gt = gp.tile([P, 512], FP, tag="g", name=f"v{g}")
                for ct in range(NCT):
                    nc.tensor.matmul(
                        gt[:], lhsT=xT[:, ct, P * g:P * g + P],
                        rhs=wv[:, ct, :],
                        start=(ct == 0), stop=(ct == NCT - 1))
                    if ct == 3:
                        yield
                nc.vector.tensor_copy(v_sb[:, g, :], gt[:])
                yield

            def proj_group(tt, nb):
                gt = gp.tile([P, 512], FP, tag="g", name=f"pj{tt}_{nb}")
                for pp in range(4):
                    nc.tensor.matmul(
                        gt[:], lhsT=ao[:, pp, P * tt:P * tt + P],
                        rhs=wpt[:, pp, 512 * nb:512 * nb + 512],
                        start=(pp == 0), stop=(pp == 3))
                yt = ytp.tile([P, 512], BF, tag="yt", name=f"yt{tt}_{nb}")
                nc.vector.tensor_copy(yt[:], gt[:])
                nc.sync.dma_start(
                    y_d.ap()[P * tt:P * tt + P, 512 * nb:512 * nb + 512], yt[:])
                yield

            # --- attention pairs ---
            for p in range(4):
                sl = p % 2
                if p == 0:
                    fill = [qk_group(0, "k", gi) for gi in range(1, 4)] \
                        + [v_group(g) for g in range(NST)] \
                        + [qk_group(1, w, gi) for w in ("q", "k") for gi in range(4)]
                    stage_w(1)
                elif p < 3:
                    fill = [qk_group(p + 1, w, gi) for w in ("q", "k") for gi in range(4)]
                    stage_w(p + 1)
                else:
                    fill = []
                bcount = 0

                def pop_fill(quanta=1):
                    # advance the front filler generator by one quantum per
                    # block; junk matmuls keep the PE's clock gate warm once
                    # real fillers dry up
                    done = 0
                    while fill and done < quanta:
                        try:
                            next(fill[0])
                            done += 1
                        except StopIteration:
                            fill.pop(0)
                    if done == 0 and bcount % 2 == 0:
                        gt = gp.tile([P, 512], FP, tag="g", name=f"jk{p}_{bcount}")
                        for _ in range(4):
                            nc.tensor.matmul(gt[:], lhsT=warm[:, :P], rhs=warm[:],
                                             start=True, stop=True)

                def av_gen(clo, chi, jmax, tts):
                    cw = chi - clo
                    gt = gp.tile([P, 512], FP, tag="g", name=f"av{p}_{clo}")
                    for j in range(jmax + 1):
                        lo2 = max(clo, P * j)
                        for hl in range(2):
                            nc.tensor.matmul(
                                gt[64 * hl:64 * hl + 64, lo2 - clo:cw],
                                lhsT=vp[:, sl, j, 64 * hl:64 * hl + 64],
                                rhs=prow[j][:, hl, lo2 - P * j:chi - P * j],
                                start=(j == 0), stop=(j == jmax))
                        if j % 4 == 3 and j != jmax:
                            yield
                    nc.vector.tensor_copy(ao[:, p, clo:chi], gt[:, 0:cw])
                    if p == 3:
                        for tt in tts:
                            fill.append(proj_group(tt, 0))
                            fill.append(proj_group(tt, 1))
                    yield

                pend_av = []

                def emit_av(clo, chi, jmax, tts, delay=1):
                    # AV bursts are queued a few blocks later (so the vp
                    # chain their last matmul waits on has drained) and go to
                    # the fill front so their psum bank is held across as few
                    # pops as possible
                    pend_av.append((bcount + delay, (clo, chi, jmax, tts)))

                def make_stats(i, z, zs, nb):
                    def emit():
                        for b in range(1, nb):
                            nc.gpsimd.tensor_add(zs[:, 0:1], zs[:, 0:1],
                                                 zs[:, b:b + 1])
                        nc.vector.reduce_sum(z[:, 0:1], prow[i][:, 0, :], axis=AX)
                        nc.gpsimd.tensor_sub(z[:, 1:2], zs[:, 0:1], z[:, 0:1])
                        rz = st.tile([P, 2], FP, tag="rz", name=f"rz{p}_{i}")
                        nc.vector.reciprocal(rz[:], z[:])
                        for hl in range(2):
                            nc.vector.tensor_scalar_mul(
                                vp[:, sl, i, 64 * hl:64 * hl + 64],
                                v_sb[:, i, 128 * p + 64 * hl:128 * p + 64 * hl + 64],
                                rz[:, hl:hl + 1])
                    return emit

                # flat block stream with 2-block score lookahead: score
                # matmuls are emitted two blocks ahead of their exp, so they
                # sit ahead of all filler work in the PE queue and the exp
                # stream never waits on filler backlog
                flat = []
                for i in range(NST):
                    for b, (lo, hi) in enumerate(blocks(i)):
                        flat.append((i, b, lo, hi))
                slots = {}

                def emit_sc(n):
                    i, b, lo, hi = flat[n]
                    t0 = P * i
                    bw = hi - lo
                    sc = next_slot(bw)
                    slots[n] = sc
                    for hl in range(2):
                        hb = 64 * hl
                        nc.tensor.matmul(
                            sc[:, hl, 0:bw],
                            lhsT=k_sl[hb:hb + 64, sl, t0:t0 + P],
                            rhs=q_sl[hb:hb + 64, sl, lo:hi],
                            start=True, stop=True)
                    if b == 0:
                        # causal mask: additive -1e30 triangle on the diagonal
                        # 128-block (block 0 always starts at t0)
                        nc.vector.tensor_add(sc[:, :, 0:128], sc[:, :, 0:128],
                                             trif[:])

                prow = {}
                ztile = {}
                pend_stats = None
                LOOK = 2
                for k in range(LOOK):
                    emit_sc(k)
                for n, (i, b, lo, hi) in enumerate(flat):
                    if n + LOOK < len(flat):
                        emit_sc(n + LOOK)
                    while pend_av and pend_av[0][0] <= bcount:
                        fill.insert(0, av_gen(*pend_av.pop(0)[1]))
                    t0 = P * i
                    if b == 0:
                        prow[i] = prp.tile([P, 2, T - t0], BF, tag=f"pr{i}",
                                           name=f"pr{p}_{i}")
                        ztile[i] = (st.tile([P, 2], FP, tag="z", name=f"z{p}_{i}"),
                                    st.tile([P, 4], FP, tag="zs", name=f"zs{p}_{i}"))
                    z, zs = ztile[i]
                    # merged two-head exp; accumulator = zh0+zh1 per partition
                    # (both heads share s on a partition)
                    nc.scalar.activation(
                        prow[i][:, :, lo - t0:hi - t0], slots.pop(n)[:],
                        EXP, scale=SCALE, accum_out=zs[:, b:b + 1])
                    last = n + 1 == len(flat) or flat[n + 1][0] != i
                    if last:
                        if pend_stats is not None:
                            pend_stats()
                        pend_stats = make_stats(i, z, zs, b + 1)
                        if i == NST - 1:
                            pend_stats()
                            pend_stats = None
                        if i in (4, 8, 12):
                            c = i // 4 - 1
                            emit_av(512 * c, 512 * c + 512, 4 * c + 3,
                                    range(4 * c, 4 * c + 4))
                        elif i == 14 and p == 3:
                            # early half of the last chunk so the output
                            # projection tail shrinks
                            emit_av(1536, 1792, 13, (12, 13))
                        elif i == 15:
                            if p == 3:
                                emit_av(1792, 2048, 15, (14, 15))
                            else:
                                emit_av(1536, 2048, 15, ())
                    pop_fill((3 if p == 3 else 2) if len(fill) > 8 else
                             (2 if len(fill) > 3 and p == 3 else 1))
                    bcount += 1
                while pend_av:
                    fill.insert(0, av_gen(*pend_av.pop(0)[1]))
                while fill:
                    try:
                        next(fill[0])
                    except StopIteration:
                        fill.pop(0)

    nc.compile()
    return nc


def _get_nc():
    if "nc" not in _CACHE:
        _CACHE["nc"] = _build_nc()
    return _CACHE["nc"]


def _in_maps(x, Wq, Wk, Wv, Wp):
    import ml_dtypes
    trif = np.tril(np.full((P, P), NEG, np.float32), -1)
    trif2 = np.concatenate([trif, trif], 1)
    maps = []
    for b in range(B):
        xT = np.ascontiguousarray(x[b].T)
        for g in range(2):
            heads = range(8 * g, 8 * g + 8)
            maps.append({
                "xt": xT.astype(ml_dtypes.bfloat16),
                "wq": np.ascontiguousarray(np.concatenate([Wq[h] for h in heads], 1)).astype(ml_dtypes.bfloat16),
                "wk": np.ascontiguousarray(np.concatenate([Wk[h] for h in heads], 1)).astype(ml_dtypes.bfloat16),
                "wv": np.ascontiguousarray(np.concatenate([Wv[h] for h in heads], 1)).astype(ml_dtypes.bfloat16),
                "wpt": np.ascontiguousarray(Wp[:, 512 * g:512 * g + 512].T).astype(ml_dtypes.bfloat16),
                "trif": np.ascontiguousarray(trif2),
            })
    return maps


def kernel(x, Wq, Wk, Wv, Wp, bp):
    from concourse.bass_utils import run_bass_kernel_spmd

    x = np.asarray(x, np.float32)
    Wq = np.asarray(Wq, np.float32)
    Wk = np.asarray(Wk, np.float32)
    Wv = np.asarray(Wv, np.float32)
    Wp = np.asarray(Wp, np.float32)
    bp = np.asarray(bp, np.float32)

    nc = _get_nc()
    res = run_bass_kernel_spmd(nc, _in_maps(x, Wq, Wk, Wv, Wp), list(range(8)))
    y = np.empty((B, T, C), np.float32)
    for b in range(B):
        y[b] = (res.results[2 * b]["y"].astype(np.float32)
                + res.results[2 * b + 1]["y"].astype(np.float32) + bp)
    return y


# revision 35
# speedup vs baseline: 1.0244x; 1.0244x over previous
"""Multi-head attention (query-axis softmax variant) on 8 Trainium2 NeuronCores.

Problem: B=4, T=2048, C=1024, H=16, Dh=64.
  q/k/v = per-head projections of x; wei = (q k^T) * C**-0.5, causal-masked;
  softmax over the QUERY axis (axis=2 of (B,H,T,S)); out = attn @ v, concat
  heads, project with Wp and add bp.

Sharding: 8 cores = 4 batches x 2 head-groups (8 heads each).  Each core
computes a partial projection output for its batch; host sums the two
group partials per batch and adds the bias.

Per-core dataflow is fully "transposed" (features on partitions, tokens on
the free axis) so the query-axis softmax stats become free-axis reductions.
Score blocks rotate through pipelined PSUM slots (A = [P,2,1024], its second
half, and B = [P,2,512]) so the next block's matmuls run while the previous
block's exp drains.  Exps are merged two-head 3D-AP ACTIVATEs whose
accumulator yields zh0+zh1 per partition (both heads share the same s on a
partition); h0's Z comes from one DVE reduce of the retained P row and h1's
by subtraction on gpsimd.  attout accumulates per 512-col chunk over the
retained P rows with col-group-paired matmuls; projections (v, q/k of the
next pair, output) fill the PE between score bursts.
"""
import numpy as np

T = 2048
C = 1024
H = 16
DH = 64
B = 4
SCALE = float(C) ** -0.5
NEG = -1e30
P = 128
NCT = 8       # c-tiles (contraction tiles of 128 over C)
NST = 16      # s-tiles of 128 over T

_CACHE = {}


def _build_nc():
    import concourse.bacc as bacc
    import concourse.tile as tile
    import concourse.mybir as mybir

    FP = mybir.dt.float32
    BF = mybir.dt.bfloat16
    AX = mybir.AxisListType.X
    EXP = mybir.ActivationFunctionType.Exp

    nc = bacc.Bacc("TRN2", target_bir_lowering=False, debug=False, num_devices=8)

    xT_d = nc.declare_dram_parameter("xt", [C, T], BF, isOutput=False)
    wq_d = nc.declare_dram_parameter("wq", [C, 512], BF, isOutput=False)
    wk_d = nc.declare_dram_parameter("wk", [C, 512], BF, isOutput=False)
    wv_d = nc.declare_dram_parameter("wv", [C, 512], BF, isOutput=False)
    wp_d = nc.declare_dram_parameter("wpt", [512, C], BF, isOutput=False)
    trif_d = nc.declare_dram_parameter("trif", [P, 256], FP, isOutput=False)
    y_d = nc.declare_dram_parameter("y", [T, C], BF, isOutput=True)

    def blocks(i):
        # all score blocks are <= 512 wide; they rotate through three
        # [P, 2, 512] psum slot units for pipeline depth 3 between a block's
        # exp and later blocks' matmuls
        t0 = P * i
        out = []
        lo = t0
        while lo < T:
            hi = min(lo + 512 - lo % 512 if lo % 512 else lo + 512, T)
            out.append((lo, hi))
            lo = hi
        return out

    with tile.TileContext(nc) as tc:
        with (
            tc.tile_pool(name="perm", bufs=1) as perm,
            tc.tile_pool(name="wst", bufs=2) as wst,
            tc.tile_pool(name="prp", bufs=1) as prp,
            tc.tile_pool(name="st", bufs=6) as st,
            tc.tile_pool(name="ytp", bufs=2) as ytp,
            tc.tile_pool(name="sca", bufs=1, space="PSUM") as sca,
            tc.tile_pool(name="scb", bufs=1, space="PSUM") as scb,
            tc.tile_pool(name="gp", bufs=2, space="PSUM") as gp,
        ):
            warm = perm.tile([P, 512], BF, tag="warm")
            nc.vector.memset(warm[:], 0.0)
            dummy = perm.tile([P, 1], BF, tag="dummy")
            # preload the exp table set off the critical path
            nc.scalar.activation(dummy[:], warm[:, 0:1], EXP, scale=1.0)

            wqt = {}
            wkt = {}

            def stage_w(pn):
                wqt[pn] = wst.tile([P, NCT, P], BF, tag="wq", name=f"wq{pn}")
                wkt[pn] = wst.tile([P, NCT, P], BF, tag="wk", name=f"wk{pn}")
                nc.sync.dma_start(
                    wqt[pn][:],
                    wq_d.ap()[:, P * pn:P * pn + P].rearrange("(a c) m -> c a m", c=P))
                nc.sync.dma_start(
                    wkt[pn][:],
                    wk_d.ap()[:, P * pn:P * pn + P].rearrange("(a c) m -> c a m", c=P))

            stage_w(0)
            xT = perm.tile([P, NCT, T], BF, tag="xT")
            for ct in range(NCT):
                nc.sync.dma_start(xT[:, ct, :], xT_d.ap()[P * ct:P * ct + P, :])
            trif = perm.tile([P, 2, 128], FP, tag="trif")
            nc.sync.dma_start(trif[:], trif_d.ap().rearrange("p (a c) -> p a c", a=2))
            wv = perm.tile([P, NCT, 512], BF, tag="wv")
            nc.sync.dma_start(wv[:], wv_d.ap().rearrange("(a c) m -> c a m", c=P))
            wpt = perm.tile([P, 4, C], BF, tag="wpt")
            nc.sync.dma_start(wpt[:], wp_d.ap().rearrange("(a c) m -> c a m", c=P))

            q_sl = perm.tile([P, 2, T], BF, tag="q")
            k_sl = perm.tile([P, 2, T], BF, tag="k")
            v_sb = perm.tile([P, NST, 512], BF, tag="v")
            ao = perm.tile([P, 4, T], BF, tag="ao")
            vp = perm.tile([P, 2, NST, 128], BF, tag="vp")

            # score slots: allocated once, rotation handled by subtile deps
            scA = sca.tile([P, 2, 1024], FP, tag="sca", name="scA")
            scB = scb.tile([P, 2, 512], FP, tag="scb", name="scB")

            def slot_view(slot, bw):
                if slot == 0:
                    return scA[:, :, 0:bw]
                if slot == 1:
                    return scA[:, :, 512:512 + bw]
                return scB[:, :, 0:bw]

            slot_cur = [0]

            def next_slot(bw):
                s = slot_cur[0]
                slot_cur[0] = (s + 1) % 3
                return slot_view(s, bw)

            # --- warmup junk (PE clock gate) while first DMAs land ---
            for _ in range(12):
                nc.tensor.matmul(scA[:, 0, 0:512], lhsT=warm[:, :P], rhs=warm[:],
                                 start=True, stop=True)

            # --- prefix: q of pair 0 (all 4 groups) + k group 0 only; the
            # remaining k groups become pair-0's first fillers (k group g is
            # first needed as lhsT at s-tile 4g)
            gA = gp.tile([P, 512], FP, tag="g", name="pre_q0")
            gB = gp.tile([P, 512], FP, tag="g", name="pre_q1")
            qdst = [gA[:, :], gB[:, :], scB[:, 0, :], scB[:, 1, :]]
            kdst0 = scA[:, 0, 0:512]
            for ct in range(NCT):
                for gi in range(4):
                    nc.tensor.matmul(
                        qdst[gi], lhsT=wqt[0][:, ct, :],
                        rhs=xT[:, ct, 512 * gi:512 * gi + 512],
                        start=(ct == 0), stop=(ct == NCT - 1))
                nc.tensor.matmul(
                    kdst0, lhsT=wkt[0][:, ct, :], rhs=xT[:, ct, 0:512],
                    start=(ct == 0), stop=(ct == NCT - 1))
            nc.vector.tensor_copy(k_sl[:, 0, 0:512], kdst0)
            for gi in range(4):
                nc.vector.tensor_copy(q_sl[:, 0, 512 * gi:512 * gi + 512], qdst[gi])

            # --- filler generators: each next() emits a ~0.85us quantum of PE
            # work (4 matmuls) so the PE interleave tracks the exp cadence ---
            def qk_group(pn, which, gi):
                gt = gp.tile([P, 512], FP, tag="g", name=f"{which}{pn}_{gi}")
                wt = wqt[pn] if which == "q" else wkt[pn]
                dst = q_sl if which == "q" else k_sl
                for ct in range(NCT):
                    nc.tensor.matmul(
                        gt[:], lhsT=wt[:, ct, :],
                        rhs=xT[:, ct, 512 * gi:512 * gi + 512],
                        start=(ct == 0), stop=(ct == NCT - 1))
                    if ct == 3:
                        yield
                nc.vector.tensor_copy(dst[:, pn % 2, 512 * gi:512 * gi + 512], gt[:])
                yield

            def v_group(g):
                gt = gp.tile([P, 512], FP, tag="g", name=f"v{g}")
                for ct in range(NCT):
                    nc.tensor.matmul(
                        gt[:], lhsT=xT[:, ct, P * g:P * g + P],
                        rhs=wv[:, ct, :],
                        start=(ct == 0), stop=(ct == NCT - 1))
                    if ct == 3:
                        yield
                nc.vector.tensor_copy(v_sb[:, g, :], gt[:])
                yield

            def proj_group(tt, nb):
                gt = gp.tile([P, 512], FP, tag="g", name=f"pj{tt}_{nb}")
                for pp in range(4):
                    nc.tensor.matmul(
                        gt[:], lhsT=ao[:, pp, P * tt:P * tt + P],
                        rhs=wpt[:, pp, 512 * nb:512 * nb + 512],
                        start=(pp == 0), stop=(pp == 3))
                yt = ytp.tile([P, 512], BF, tag="yt", name=f"yt{tt}_{nb}")
                nc.vector.tensor_copy(yt[:], gt[:])
                nc.sync.dma_start(
                    y_d.ap()[P * tt:P * tt + P, 512 * nb:512 * nb + 512], yt[:])
                yield

            # --- attention pairs ---
            for p in range(4):
                sl = p % 2
                if p == 0:
                    fill = [qk_group(0, "k", gi) for gi in range(1, 4)] \
                        + [v_group(g) for g in range(NST)] \
                        + [qk_group(1, w, gi) for w in ("q", "k") for gi in range(4)]
                    stage_w(1)
                elif p < 3:
                    fill = [qk_group(p + 1, w, gi) for w in ("q", "k") for gi in range(4)]
                    stage_w(p + 1)
                else:
                    fill = []
                bcount = 0

                def pop_fill(quanta=1):
                    # advance the front filler generator by one quantum per
                    # block; junk matmuls keep the PE's clock gate warm once
                    # real fillers dry up
                    done = 0
                    while fill and done < quanta:
                        try:
                            next(fill[0])
                            done += 1
                        except StopIteration:
                            fill.pop(0)
                    if done == 0 and bcount % 2 == 0:
                        gt = gp.tile([P, 512], FP, tag="g", name=f"jk{p}_{bcount}")
                        for _ in range(4):
                            nc.tensor.matmul(gt[:], lhsT=warm[:, :P], rhs=warm[:],
                                             start=True, stop=True)

                def av_gen(clo, chi, jmax, tts):
                    cw = chi - clo
                    gt = gp.tile([P, 512], FP, tag="g", name=f"av{p}_{clo}")
                    for j in range(jmax + 1):
                        lo2 = max(clo, P * j)
                        for hl in range(2):
                            nc.tensor.matmul(
                                gt[64 * hl:64 * hl + 64, lo2 - clo:cw],
                                lhsT=vp[:, sl, j, 64 * hl:64 * hl + 64],
                                rhs=prow[j][:, hl, lo2 - P * j:chi - P * j],
                                start=(j == 0), stop=(j == jmax))
                        if j % 4 == 3 and j != jmax:
                            yield
                    nc.vector.tensor_copy(ao[:, p, clo:chi], gt[:, 0:cw])
                    if p == 3:
                        for tt in tts:
                            fill.append(proj_group(tt, 0))
                            fill.append(proj_group(tt, 1))
                    yield

                pend_av = []

                def emit_av(clo, chi, jmax, tts, delay=1):
                    # AV bursts are queued a few blocks later (so the vp
                    # chain their last matmul waits on has drained) and go to
                    # the fill front so their psum bank is held across as few
                    # pops as possible
                    pend_av.append((bcount + delay, (clo, chi, jmax, tts)))

                def make_stats(i, z, zs, nb):
                    def emit():
                        for b in range(1, nb):
                            nc.gpsimd.tensor_add(zs[:, 0:1], zs[:, 0:1],
                                                 zs[:, b:b + 1])
                        nc.vector.reduce_sum(z[:, 0:1], prow[i][:, 0, :], axis=AX)
                        nc.gpsimd.tensor_sub(z[:, 1:2], zs[:, 0:1], z[:, 0:1])
                        rz = st.tile([P, 2], FP, tag="rz", name=f"rz{p}_{i}")
                        nc.vector.reciprocal(rz[:], z[:])
                        for hl in range(2):
                            nc.vector.tensor_scalar_mul(
                                vp[:, sl, i, 64 * hl:64 * hl + 64],
                                v_sb[:, i, 128 * p + 64 * hl:128 * p + 64 * hl + 64],
                                rz[:, hl:hl + 1])
                    return emit

                # flat block stream with 2-block score lookahead: score
                # matmuls are emitted two blocks ahead of their exp, so they
                # sit ahead of all filler work in the PE queue and the exp
                # stream never waits on filler backlog
                flat = []
                for i in range(NST):
                    for b, (lo, hi) in enumerate(blocks(i)):
                        flat.append((i, b, lo, hi))
                slots = {}

                def emit_sc(n):
                    i, b, lo, hi = flat[n]
                    t0 = P * i
                    bw = hi - lo
                    sc = next_slot(bw)
                    slots[n] = sc
                    for hl in range(2):
                        hb = 64 * hl
                        nc.tensor.matmul(
                            sc[:, hl, 0:bw],
                            lhsT=k_sl[hb:hb + 64, sl, t0:t0 + P],
                            rhs=q_sl[hb:hb + 64, sl, lo:hi],
                            start=True, stop=True)
                    if b == 0:
                        # causal mask: additive -1e30 triangle on the diagonal
                        # 128-block (block 0 always starts at t0)
                        nc.vector.tensor_add(sc[:, :, 0:128], sc[:, :, 0:128],
                                             trif[:])

                prow = {}
                ztile = {}
                pend_stats = None
                LOOK = 2
                for k in range(LOOK):
                    emit_sc(k)
                for n, (i, b, lo, hi) in enumerate(flat):
                    if n + LOOK < len(flat):
                        emit_sc(n + LOOK)
                    while pend_av and pend_av[0][0] <= bcount:
                        fill.insert(0, av_gen(*pend_av.pop(0)[1]))
                    t0 = P * i
                    if b == 0:
                        prow[i] = prp.tile([P, 2, T - t0], BF, tag=f"pr{i}",
                                           name=f"pr{p}_{i}")
                        ztile[i] = (st.tile([P, 2], FP, tag="z", name=f"z{p}_{i}"),
                                    st.tile([P, 4], FP, tag="zs", name=f"zs{p}_{i}"))
                    z, zs = ztile[i]
                    # merged two-head exp; accumulator = zh0+zh1 per partition
                    # (both heads share s on a partition)
                    nc.scalar.activation(
                        prow[i][:, :, lo - t0:hi - t0], slots.pop(n)[:],
                        EXP, scale=SCALE, accum_out=zs[:, b:b + 1])
                    last = n + 1 == len(flat) or flat[n + 1][0] != i
                    if last:
                        if pend_stats is not None:
                            pend_stats()
                        pend_stats = make_stats(i, z, zs, b + 1)
                        if i == NST - 1:
                            pend_stats()
                            pend_stats = None
                        if i in (4, 8, 12):
                            c = i // 4 - 1
                            emit_av(512 * c, 512 * c + 512, 4 * c + 3,
                                    range(4 * c, 4 * c + 4))
                        elif i == 14 and p == 3:
                            # early half of the last chunk so the output
                            # projection tail shrinks
                            emit_av(1536, 1792, 13, (12, 13))
                        elif i == 15:
                            if p == 3:
                                emit_av(1792, 2048, 15, (14, 15))
                            else:
                                emit_av(1536, 2048, 15, ())
                    pop_fill(2 if len(fill) > (3 if p == 3 else 8) else 1)
                    bcount += 1
                while pend_av:
                    fill.insert(0, av_gen(*pend_av.pop(0)[1]))
                while fill:
                    try:
                        next(fill[0])
                    except StopIteration:
                        fill.pop(0)

    nc.compile()
    return nc


def _get_nc():
    if "nc" not in _CACHE:
        _CACHE["nc"] = _build_nc()
    return _CACHE["nc"]


def _in_maps(x, Wq, Wk, Wv, Wp):
    import ml_dtypes
    trif = np.tril(np.full((P, P), NEG, np.float32), -1)
    trif2 = np.concatenate([trif, trif], 1)
    maps = []
    for b in range(B):
        xT = np.ascontiguousarray(x[b].T)
        for g in range(2):
            heads = range(8 * g, 8 * g + 8)
            maps.append({
                "xt": xT.astype(ml_dtypes.bfloat16),
                "wq": np.ascontiguousarray(np.concatenate([Wq[h] for h in heads], 1)).astype(ml_dtypes.bfloat16),
                "wk": np.ascontiguousarray(np.concatenate([Wk[h] for h in heads], 1)).astype(ml_dtypes.bfloat16),
                "wv": np.ascontiguousarray(np.concatenate([Wv[h] for h in heads], 1)).astype(ml_dtypes.bfloat16),
                "wpt": np.ascontiguousarray(Wp[:, 512 * g:512 * g + 512].T).astype(ml_dtypes.bfloat16),
                "trif": np.ascontiguousarray(trif2),
            })
    return maps


def kernel(x, Wq, Wk, Wv, Wp, bp):
    from concourse.bass_utils import run_bass_kernel_spmd

    x = np.asarray(x, np.float32)
    Wq = np.asarray(Wq, np.float32)
    Wk = np.asarray(Wk, np.float32)
    Wv = np.asarray(Wv, np.float32)
    Wp = np.asarray(Wp, np.float32)
    bp = np.asarray(bp, np.float32)

    nc = _get_nc()
    res = run_bass_kernel_spmd(nc, _in_maps(x, Wq, Wk, Wv, Wp), list(range(8)))
    y = np.empty((B, T, C), np.float32)
    for b in range(B):
        y[b] = (res.results[2 * b]["y"].astype(np.float32)
                + res.results[2 * b + 1]["y"].astype(np.float32) + bp)
    return y


# revision 36
# speedup vs baseline: 1.2152x; 1.1863x over previous
"""Multi-head attention (query-axis softmax variant) on 8 Trainium2 NeuronCores.

Problem: B=4, T=2048, C=1024, H=16, Dh=64.
  q/k/v = per-head projections of x; wei = (q k^T) * C**-0.5, causal-masked;
  softmax over the QUERY axis (axis=2 of (B,H,T,S)); out = attn @ v, concat
  heads, project with Wp and add bp.

Sharding: 8 cores = 4 batches x 2 head-groups (8 heads each).  Each core
computes a partial projection output for its batch; host sums the two
group partials per batch and adds the bias.

Per-core dataflow is fully "transposed" (features on partitions, tokens on
the free axis) so the query-axis softmax stats become free-axis reductions.
Score blocks rotate through pipelined PSUM slots (A = [P,2,1024], its second
half, and B = [P,2,512]) so the next block's matmuls run while the previous
block's exp drains.  Exps are merged two-head 3D-AP ACTIVATEs whose
accumulator yields zh0+zh1 per partition (both heads share the same s on a
partition); h0's Z comes from one DVE reduce of the retained P row and h1's
by subtraction on gpsimd.  attout accumulates per 512-col chunk over the
retained P rows with col-group-paired matmuls; projections (v, q/k of the
next pair, output) fill the PE between score bursts.
"""
import numpy as np

T = 2048
C = 1024
H = 16
DH = 64
B = 4
SCALE = float(C) ** -0.5
NEG = -1e30
P = 128
NCT = 8       # c-tiles (contraction tiles of 128 over C)
NST = 16      # s-tiles of 128 over T

_CACHE = {}


def _build_nc():
    import concourse.bacc as bacc
    import concourse.tile as tile
    import concourse.mybir as mybir

    FP = mybir.dt.float32
    BF = mybir.dt.bfloat16
    AX = mybir.AxisListType.X
    EXP = mybir.ActivationFunctionType.Exp

    nc = bacc.Bacc("TRN2", target_bir_lowering=False, debug=False, num_devices=8)

    xT_d = nc.declare_dram_parameter("xt", [C, T], BF, isOutput=False)
    wq_d = nc.declare_dram_parameter("wq", [C, 512], BF, isOutput=False)
    wk_d = nc.declare_dram_parameter("wk", [C, 512], BF, isOutput=False)
    wv_d = nc.declare_dram_parameter("wv", [C, 512], BF, isOutput=False)
    wp_d = nc.declare_dram_parameter("wpt", [512, C], BF, isOutput=False)
    trif_d = nc.declare_dram_parameter("trif", [P, 256], FP, isOutput=False)
    y_d = nc.declare_dram_parameter("y", [T, C], BF, isOutput=True)

    def blocks(i):
        # all score blocks are <= 512 wide; they rotate through three
        # [P, 2, 512] psum slot units for pipeline depth 3 between a block's
        # exp and later blocks' matmuls
        t0 = P * i
        out = []
        lo = t0
        while lo < T:
            hi = min(lo + 512 - lo % 512 if lo % 512 else lo + 512, T)
            out.append((lo, hi))
            lo = hi
        return out

    with tile.TileContext(nc) as tc:
        with (
            tc.tile_pool(name="perm", bufs=1) as perm,
            tc.tile_pool(name="wst", bufs=2) as wst,
            tc.tile_pool(name="prp", bufs=1) as prp,
            tc.tile_pool(name="st", bufs=6) as st,
            tc.tile_pool(name="ytp", bufs=2) as ytp,
            tc.tile_pool(name="sca", bufs=1, space="PSUM") as sca,
            tc.tile_pool(name="scb", bufs=1, space="PSUM") as scb,
            tc.tile_pool(name="gp", bufs=2, space="PSUM") as gp,
        ):
            warm = perm.tile([P, 512], BF, tag="warm")
            nc.vector.memset(warm[:], 0.0)
            dummy = perm.tile([P, 1], BF, tag="dummy")
            # preload the exp table set off the critical path
            nc.scalar.activation(dummy[:], warm[:, 0:1], EXP, scale=1.0)

            wqt = {}
            wkt = {}

            def stage_w(pn):
                wqt[pn] = wst.tile([P, NCT, P], BF, tag="wq", name=f"wq{pn}")
                wkt[pn] = wst.tile([P, NCT, P], BF, tag="wk", name=f"wk{pn}")
                nc.sync.dma_start(
                    wqt[pn][:],
                    wq_d.ap()[:, P * pn:P * pn + P].rearrange("(a c) m -> c a m", c=P))
                nc.sync.dma_start(
                    wkt[pn][:],
                    wk_d.ap()[:, P * pn:P * pn + P].rearrange("(a c) m -> c a m", c=P))

            stage_w(0)
            xT = perm.tile([P, NCT, T], BF, tag="xT")
            for ct in range(NCT):
                nc.sync.dma_start(xT[:, ct, :], xT_d.ap()[P * ct:P * ct + P, :])
            trif = perm.tile([P, 2, 128], FP, tag="trif")
            nc.sync.dma_start(trif[:], trif_d.ap().rearrange("p (a c) -> p a c", a=2))
            wv = perm.tile([P, NCT, 512], BF, tag="wv")
            nc.sync.dma_start(wv[:], wv_d.ap().rearrange("(a c) m -> c a m", c=P))
            wpt = perm.tile([P, 4, C], BF, tag="wpt")
            nc.sync.dma_start(wpt[:], wp_d.ap().rearrange("(a c) m -> c a m", c=P))

            q_sl = perm.tile([P, 2, T], BF, tag="q")
            k_sl = perm.tile([P, 2, T], BF, tag="k")
            v_sb = perm.tile([P, NST, 512], BF, tag="v")
            ao = perm.tile([P, 4, T], BF, tag="ao")
            vp = perm.tile([P, 2, NST, 128], BF, tag="vp")

            # score slots: allocated once, rotation handled by subtile deps
            scA = sca.tile([P, 2, 1024], FP, tag="sca", name="scA")
            scB = scb.tile([P, 2, 512], FP, tag="scb", name="scB")

            def slot_view(slot, bw):
                if slot == 0:
                    return scA[:, :, 0:bw]
                if slot == 1:
                    return scA[:, :, 512:512 + bw]
                return scB[:, :, 0:bw]

            slot_cur = [0]

            def next_slot(bw):
                s = slot_cur[0]
                slot_cur[0] = (s + 1) % 3
                return slot_view(s, bw)

            # --- warmup junk (PE clock gate) while first DMAs land ---
            for _ in range(12):
                nc.tensor.matmul(scA[:, 0, 0:512], lhsT=warm[:, :P], rhs=warm[:],
                                 start=True, stop=True)

            # --- prefix: q of pair 0 (all 4 groups) + k group 0 only; the
            # remaining k groups become pair-0's first fillers (k group g is
            # first needed as lhsT at s-tile 4g)
            gA = gp.tile([P, 512], FP, tag="g", name="pre_q0")
            gB = gp.tile([P, 512], FP, tag="g", name="pre_q1")
            qdst = [gA[:, :], gB[:, :], scB[:, 0, :], scB[:, 1, :]]
            kdst0 = scA[:, 0, 0:512]
            for ct in range(NCT):
                for gi in range(4):
                    nc.tensor.matmul(
                        qdst[gi], lhsT=wqt[0][:, ct, :],
                        rhs=xT[:, ct, 512 * gi:512 * gi + 512],
                        start=(ct == 0), stop=(ct == NCT - 1))
                nc.tensor.matmul(
                    kdst0, lhsT=wkt[0][:, ct, :], rhs=xT[:, ct, 0:512],
                    start=(ct == 0), stop=(ct == NCT - 1))
            nc.vector.tensor_copy(k_sl[:, 0, 0:512], kdst0)
            for gi in range(4):
                nc.vector.tensor_copy(q_sl[:, 0, 512 * gi:512 * gi + 512], qdst[gi])

            # --- filler generators: each next() emits a ~0.85us quantum of PE
            # work (4 matmuls) so the PE interleave tracks the exp cadence ---
            def qk_group(pn, which, gi):
                gt = gp.tile([P, 512], FP, tag="g", name=f"{which}{pn}_{gi}")
                wt = wqt[pn] if which == "q" else wkt[pn]
                dst = q_sl if which == "q" else k_sl
                for ct in range(NCT):
                    nc.tensor.matmul(
                        gt[:], lhsT=wt[:, ct, :],
                        rhs=xT[:, ct, 512 * gi:512 * gi + 512],
                        start=(ct == 0), stop=(ct == NCT - 1))
                    if ct == 3:
                        yield
                nc.vector.tensor_copy(dst[:, pn % 2, 512 * gi:512 * gi + 512], gt[:])
                yield

            def v_group(g):
                gt = gp.tile([P, 512], FP, tag="g", name=f"v{g}")
                for ct in range(NCT):
                    nc.tensor.matmul(
                        gt[:], lhsT=xT[:, ct, P * g:P * g + P],
                        rhs=wv[:, ct, :],
                        start=(ct == 0), stop=(ct == NCT - 1))
                    if ct == 3:
                        yield
                nc.vector.tensor_copy(v_sb[:, g, :], gt[:])
                yield

            def proj_group(tt, nb):
                gt = gp.tile([P, 512], FP, tag="g", name=f"pj{tt}_{nb}")
                for pp in range(4):
                    nc.tensor.matmul(
                        gt[:], lhsT=ao[:, pp, P * tt:P * tt + P],
                        rhs=wpt[:, pp, 512 * nb:512 * nb + 512],
                        start=(pp == 0), stop=(pp == 3))
                yt = ytp.tile([P, 512], BF, tag="yt", name=f"yt{tt}_{nb}")
                nc.vector.tensor_copy(yt[:], gt[:])
                nc.sync.dma_start(
                    y_d.ap()[P * tt:P * tt + P, 512 * nb:512 * nb + 512], yt[:])
                yield

            # --- attention pairs ---
            for p in range(4):
                sl = p % 2
                if p == 0:
                    fill = [qk_group(0, "k", gi) for gi in range(1, 4)] \
                        + [v_group(g) for g in range(NST)] \
                        + [qk_group(1, w, gi) for w in ("q", "k") for gi in range(4)]
                    stage_w(1)
                elif p < 3:
                    fill = [qk_group(p + 1, w, gi) for w in ("q", "k") for gi in range(4)]
                    stage_w(p + 1)
                else:
                    fill = []
                bcount = 0

                def pop_fill(quanta=1):
                    # advance the front filler generator by one quantum per
                    # block; junk matmuls keep the PE's clock gate warm once
                    # real fillers dry up
                    done = 0
                    while fill and done < quanta:
                        try:
                            next(fill[0])
                            done += 1
                        except StopIteration:
                            fill.pop(0)
                    if done == 0 and bcount % 2 == 0:
                        gt = gp.tile([P, 512], FP, tag="g", name=f"jk{p}_{bcount}")
                        for _ in range(4):
                            nc.tensor.matmul(gt[:], lhsT=warm[:, :P], rhs=warm[:],
                                             start=True, stop=True)

                def av_gen(clo, chi, jmax, tts):
                    cw = chi - clo
                    gt = gp.tile([P, 512], FP, tag="g", name=f"av{p}_{clo}")
                    for j in range(jmax + 1):
                        lo2 = max(clo, P * j)
                        for hl in range(2):
                            nc.tensor.matmul(
                                gt[64 * hl:64 * hl + 64, lo2 - clo:cw],
                                lhsT=vp[:, sl, j, 64 * hl:64 * hl + 64],
                                rhs=prow[j][:, hl, lo2 - P * j:chi - P * j],
                                start=(j == 0), stop=(j == jmax))
                        if j % 4 == 3 and j != jmax:
                            yield
                    nc.vector.tensor_copy(ao[:, p, clo:chi], gt[:, 0:cw])
                    if p == 3:
                        for tt in tts:
                            fill.append(proj_group(tt, 0))
                            fill.append(proj_group(tt, 1))
                    yield

                pend_av = []

                def emit_av(clo, chi, jmax, tts, delay=1):
                    # AV bursts are queued a few blocks later (so the vp
                    # chain their last matmul waits on has drained) and go to
                    # the fill front so their psum bank is held across as few
                    # pops as possible
                    pend_av.append((bcount + delay, (clo, chi, jmax, tts)))

                def make_stats(i, z, zs, nb):
                    def emit():
                        for b in range(1, nb):
                            nc.gpsimd.tensor_add(zs[:, 0:1], zs[:, 0:1],
                                                 zs[:, b:b + 1])
                        nc.vector.reduce_sum(z[:, 0:1], prow[i][:, 0, :], axis=AX)
                        nc.gpsimd.tensor_sub(z[:, 1:2], zs[:, 0:1], z[:, 0:1])
                        rz = st.tile([P, 2], FP, tag="rz", name=f"rz{p}_{i}")
                        nc.vector.reciprocal(rz[:], z[:])
                        for hl in range(2):
                            nc.vector.tensor_scalar_mul(
                                vp[:, sl, i, 64 * hl:64 * hl + 64],
                                v_sb[:, i, 128 * p + 64 * hl:128 * p + 64 * hl + 64],
                                rz[:, hl:hl + 1])
                    return emit

                # flat block stream with 2-block score lookahead: score
                # matmuls are emitted two blocks ahead of their exp, so they
                # sit ahead of all filler work in the PE queue and the exp
                # stream never waits on filler backlog
                flat = []
                for i in range(NST):
                    for b, (lo, hi) in enumerate(blocks(i)):
                        flat.append((i, b, lo, hi))
                slots = {}

                def emit_sc(n):
                    i, b, lo, hi = flat[n]
                    t0 = P * i
                    bw = hi - lo
                    sc = next_slot(bw)
                    slots[n] = sc
                    for hl in range(2):
                        hb = 64 * hl
                        nc.tensor.matmul(
                            sc[:, hl, 0:bw],
                            lhsT=k_sl[hb:hb + 64, sl, t0:t0 + P],
                            rhs=q_sl[hb:hb + 64, sl, lo:hi],
                            start=True, stop=True)
                    if b == 0:
                        # causal mask: additive -1e30 triangle on the diagonal
                        # 128-block (block 0 always starts at t0)
                        nc.vector.tensor_add(sc[:, :, 0:128], sc[:, :, 0:128],
                                             trif[:])

                prow = {}
                ztile = {}
                pend_stats = None
                LOOK = 2
                for k in range(LOOK):
                    emit_sc(k)
                for n, (i, b, lo, hi) in enumerate(flat):
                    if n + LOOK < len(flat):
                        emit_sc(n + LOOK)
                    while pend_av and pend_av[0][0] <= bcount:
                        fill.insert(0, av_gen(*pend_av.pop(0)[1]))
                    t0 = P * i
                    if b == 0:
                        prow[i] = prp.tile([P, 2, T - t0], BF, tag=f"pr{i}",
                                           name=f"pr{p}_{i}")
                        ztile[i] = (st.tile([P, 2], FP, tag="z", name=f"z{p}_{i}"),
                                    st.tile([P, 4], FP, tag="zs", name=f"zs{p}_{i}"))
                    z, zs = ztile[i]
                    # merged two-head exp; accumulator = zh0+zh1 per partition
                    # (both heads share s on a partition)
                    nc.scalar.activation(
                        prow[i][:, :, lo - t0:hi - t0], slots.pop(n)[:],
                        EXP, scale=SCALE, accum_out=zs[:, b:b + 1])
                    last = n + 1 == len(flat) or flat[n + 1][0] != i
                    if last:
                        if pend_stats is not None:
                            pend_stats()
                        pend_stats = make_stats(i, z, zs, b + 1)
                        if i == NST - 1:
                            pend_stats()
                            pend_stats = None
                        if i in (4, 8, 12):
                            c = i // 4 - 1
                            emit_av(512 * c, 512 * c + 512, 4 * c + 3,
                                    range(4 * c, 4 * c + 4))
                        elif i == 14 and p == 3:
                            # early half of the last chunk so the output
                            # projection tail shrinks
                            emit_av(1536, 1792, 13, (12, 13))
                        elif i == 15:
                            if p == 3:
                                emit_av(1792, 2048, 15, (14, 15))
                            else:
                                emit_av(1536, 2048, 15, ())
                    pop_fill(2 if len(fill) > 8 else 1)
                    bcount += 1
                while pend_av:
                    fill.insert(0, av_gen(*pend_av.pop(0)[1]))
                while fill:
                    try:
                        next(fill[0])
                    except StopIteration:
                        fill.pop(0)

    nc.compile()
    return nc


def _get_nc():
    if "nc" not in _CACHE:
        _CACHE["nc"] = _build_nc()
    return _CACHE["nc"]


def _in_maps(x, Wq, Wk, Wv, Wp):
    import ml_dtypes
    trif = np.tril(np.full((P, P), NEG, np.float32), -1)
    trif2 = np.concatenate([trif, trif], 1)
    maps = []
    for b in range(B):
        xT = np.ascontiguousarray(x[b].T)
        for g in range(2):
            heads = range(8 * g, 8 * g + 8)
            maps.append({
                "xt": xT.astype(ml_dtypes.bfloat16),
                "wq": np.ascontiguousarray(np.concatenate([Wq[h] for h in heads], 1)).astype(ml_dtypes.bfloat16),
                "wk": np.ascontiguousarray(np.concatenate([Wk[h] for h in heads], 1)).astype(ml_dtypes.bfloat16),
                "wv": np.ascontiguousarray(np.concatenate([Wv[h] for h in heads], 1)).astype(ml_dtypes.bfloat16),
                "wpt": np.ascontiguousarray(Wp[:, 512 * g:512 * g + 512].T).astype(ml_dtypes.bfloat16),
                "trif": np.ascontiguousarray(trif2),
            })
    return maps


def kernel(x, Wq, Wk, Wv, Wp, bp):
    from concourse.bass_utils import run_bass_kernel_spmd

    x = np.asarray(x, np.float32)
    Wq = np.asarray(Wq, np.float32)
    Wk = np.asarray(Wk, np.float32)
    Wv = np.asarray(Wv, np.float32)
    Wp = np.asarray(Wp, np.float32)
    bp = np.asarray(bp, np.float32)

    nc = _get_nc()
    res = run_bass_kernel_spmd(nc, _in_maps(x, Wq, Wk, Wv, Wp), list(range(8)))
    y = np.empty((B, T, C), np.float32)
    for b in range(B):
        y[b] = (res.results[2 * b]["y"].astype(np.float32)
                + res.results[2 * b + 1]["y"].astype(np.float32) + bp)
    return y
